# revision 1
# baseline (speedup 1.0000x reference)
"""Trainium2 Bass kernel: batch-512 LSTM (H=64, D=128, T=1024) + tanh decoder.

Strategy: data-parallel over batch across 8 NeuronCores (64 rows each).
Per core, transposed-state layout: state hT/c are [H, B] tiles, gates land in
one PSUM bank [128, 2B] (col-half 0 = (f,i), col-half 1 = (o,j)); one sigmoid
activation covers f/i/o (the j-quadrant sigmoid output is unused) and a second
small activation computes tanh(j) straight from PSUM — both live in the same
ACT table set so there is a single table load. Biases ride in via an augmented
ones-row on the h-side matmul (K=65). The decoder matmul for step t is emitted
after step t+1's h-matmuls so it stays off the recurrence critical path;
decoder outputs accumulate 32 steps per PSUM bank, then one batched tanh
writes the f32 staging tile. Input x is transposed to [D, t, B] by a single
per-chunk DMA-xbar transpose (dma_start_transpose) straight from DRAM (bf16),
costing no compute-engine time. All recurrence elementwise runs in bf16
(verified end-to-end rel err ~8e-3 vs the f32 reference).
Measured on silicon: ~2.59us/step steady state, 2.70ms total, no stalls.
"""
import sys

sys.path.insert(0, "/opt/trn_rl_repo")

import numpy as np
import ml_dtypes

import concourse.bass as bass
import concourse.bacc as bacc
import concourse.mybir as mybir
from concourse.tile import TileContext
from concourse.bass_utils import run_bass_kernel_spmd

BF16 = ml_dtypes.bfloat16
F32 = mybir.dt.float32
FB = mybir.dt.bfloat16
AF = mybir.ActivationFunctionType
OP = mybir.AluOpType

B, T, D, H, A = 512, 1024, 128, 64, 16
NCORES = 8
BL = B // NCORES  # 64 batch rows per core
TC = 128          # timesteps per chunk
DEC_BLK = 32      # timesteps per decoder PSUM bank (32*16 = 512 f32 = 1 bank)

C_DT = FB         # cell-state dtype (bf16 verified: end-to-end rel err ~8e-3)
NG = 1            # interleaved batch groups per core (latency hiding)
GP_OFFLOAD = False # run m1/v4 on GpSimd to unload the vector engine


def build_nc(t_total=T):
    nc = bacc.Bacc()
    obss = nc.declare_dram_parameter("obss", [BL, T, D], FB, isOutput=False)
    wxif_d = nc.declare_dram_parameter("wxif", [D, 2 * H], FB, isOutput=False)
    wxjo_d = nc.declare_dram_parameter("wxjo", [D, 2 * H], FB, isOutput=False)
    whbif_d = nc.declare_dram_parameter("whbif", [H + 1, 2 * H], FB, isOutput=False)
    whbjo_d = nc.declare_dram_parameter("whbjo", [H + 1, 2 * H], FB, isOutput=False)
    decwb_d = nc.declare_dram_parameter("decwb", [H + 1, A], FB, isOutput=False)
    out = nc.declare_dram_parameter("out", [BL, T, A], F32, isOutput=True)

    with TileContext(nc) as tc:
        with (
            tc.tile_pool(name="const", bufs=1) as cpool,
            tc.tile_pool(name="state", bufs=1) as spool,
            tc.tile_pool(name="xT", bufs=2) as xpool,
            tc.tile_pool(name="stage", bufs=2) as stpool,
            tc.tile_pool(name="work", bufs=3) as wpool,
            tc.tile_pool(name="psz", bufs=2, space="PSUM") as pzpool,
            tc.tile_pool(name="psd", bufs=2, space="PSUM") as pdpool,
        ):
            # all tiles allocated 128-partition so every base partition is 0
            # (2-input DVE ops require equal input base partitions)
            wxif = cpool.tile([D, 2 * H], FB, tag="wxif")
            wxjo = cpool.tile([D, 2 * H], FB, tag="wxjo")
            whbif_t = cpool.tile([D, 2 * H], FB, tag="whbif")
            whbjo_t = cpool.tile([D, 2 * H], FB, tag="whbjo")
            decwb_t = cpool.tile([D, A], FB, tag="decwb")
            whbif = whbif_t[0 : H + 1, :]
            whbjo = whbjo_t[0 : H + 1, :]
            decwb = decwb_t[0 : H + 1, :]
            nc.sync.dma_start(wxif[:, :], wxif_d[:, :])
            nc.sync.dma_start(wxjo[:, :], wxjo_d[:, :])
            nc.sync.dma_start(whbif, whbif_d[:, :])
            nc.sync.dma_start(whbjo, whbjo_d[:, :])
            nc.sync.dma_start(decwb, decwb_d[:, :])

            BG = BL // NG  # batch rows per group
            hTs, csts = [], []
            for g in range(NG):
                hT_t = spool.tile([D, BG], FB, tag=f"hT{g}")
                cst_t = spool.tile([D, BG], C_DT, tag=f"c{g}")
                nc.vector.memset(hT_t[0:H, :], 0.0)
                nc.vector.memset(hT_t[H : H + 1, :], 1.0)
                nc.vector.memset(cst_t[0:H, :], 0.0)
                hTs.append(hT_t)
                csts.append(cst_t)

            n_chunks = t_total // TC
            dec_state = {}

            for ch in range(n_chunks):
                t0 = ch * TC
                xT = xpool.tile([D, TC * BL], FB, tag="xT")
                nc.sync.dma_start_transpose(
                    xT[:, :].rearrange("d (t b) -> d t b", t=TC),
                    obss[:, t0 : t0 + TC, :].rearrange("b t d -> b (t d)"),
                )
                stage = stpool.tile([BL, TC * A], F32, tag="stage")
                stage_ref = {"stage": stage}

                def emit_dec(td, g, stage_ref=stage_ref):
                    # decoder for step td, group g: out[b, A] = h @ dec_w + dec_b
                    # via the ones-row of hT; batched tanh every DEC_BLK steps
                    if td < 0:
                        return
                    dcol = td % DEC_BLK
                    if dcol == 0 and g == 0:
                        psd_tile = pdpool.tile([BL, DEC_BLK * A], F32, tag="psd")
                        dec_state["psd"] = psd_tile
                    psd = dec_state["psd"]
                    nc.tensor.matmul(
                        psd[g * BG : (g + 1) * BG, dcol * A : (dcol + 1) * A],
                        hTs[g][0 : H + 1, :], decwb, start=True, stop=True,
                    )
                    if dcol == DEC_BLK - 1 and g == NG - 1:
                        blk = td // DEC_BLK
                        nc.scalar.activation(
                            stage_ref["stage"][:, blk * DEC_BLK * A : (blk + 1) * DEC_BLK * A],
                            psd[:, :], AF.Tanh,
                        )

                for tt in range(TC):
                    for g in range(NG):
                        hT = hTs[g][0 : H + 1, :]
                        cst = csts[g][0:H, :]
                        xcol = xT[:, tt * BL + g * BG : tt * BL + (g + 1) * BG]
                        # two PSUM banks: the (f,i) sigmoid only waits for the
                        # if-half matmuls, starting ~1 matmul earlier; sigma(o)
                        # comes last and is off the critical path (only v4 needs it)
                        psz_if = pzpool.tile([2 * H, BG], F32, tag=f"pszif{g}")
                        psz_jo = pzpool.tile([2 * H, BG], F32, tag=f"pszjo{g}")
                        nc.tensor.matmul(psz_if[:, :], wxif[:, :], xcol, start=True, stop=False)
                        nc.tensor.matmul(psz_if[:, :], whbif, hT, start=False, stop=True)
                        nc.tensor.matmul(psz_jo[:, :], wxjo[:, :], xcol, start=True, stop=False)
                        nc.tensor.matmul(psz_jo[:, :], whbjo, hT, start=False, stop=True)

                        # decoder for the PREVIOUS step, emitted here so the PE
                        # runs it after this step's h-matmuls -> off the chain
                        emit_dec(tt - 1, g)

                        # gate partition layout: if-bank rows = (f; i), jo-bank
                        # rows = (o; j) — f/o at base partition 0, i/j at base 64,
                        # so every 2-input DVE op pairs operands with equal bases
                        s = wpool.tile([2 * H, BG], FB, tag=f"s{g}")
                        nc.scalar.activation(s[:, :], psz_if[:, :], AF.Sigmoid)
                        tj_t = wpool.tile([D, BG], FB, tag=f"tj{g}")
                        tj = tj_t[H : 2 * H, :]
                        nc.scalar.activation(tj, psz_jo[H : 2 * H, :], AF.Tanh)
                        so_t = wpool.tile([D, BG], FB, tag=f"so{g}")
                        so = so_t[0:H, :]
                        nc.scalar.activation(so, psz_jo[0:H, :], AF.Sigmoid)
                        sf = s[0:H, :]
                        si = s[H : 2 * H, :]

                        cf_t = wpool.tile([D, BG], C_DT, tag=f"cf{g}")
                        cf = cf_t[0:H, :]
                        nc.vector.tensor_mul(cf, cst, sf)
                        u_t = wpool.tile([D, BG], FB, tag=f"u{g}")
                        u = u_t[0:H, :]
                        nc.vector.tensor_mul(u, tj, si)
                        nc.vector.tensor_add(cst, cf, u)
                        tch_t = wpool.tile([D, BG], FB, tag=f"tch{g}")
                        tch = tch_t[0:H, :]
                        nc.scalar.activation(tch, cst, AF.Tanh)
                        (nc.gpsimd if GP_OFFLOAD else nc.vector).tensor_mul(hTs[g][0:H, :], tch, so)
                for g in range(NG):
                    emit_dec(TC - 1, g)
                nc.sync.dma_start(out[:, t0 : t0 + TC, :], stage[:, :])
    nc.finalize()
    return nc


def prep_weights(lstm_kernel, lstm_bias, dec_w, dec_b):
    K = np.asarray(lstm_kernel, np.float32)
    b = np.asarray(lstm_bias, np.float32).copy()
    i_s, j_s, f_s, o_s = (slice(0, H), slice(H, 2 * H), slice(2 * H, 3 * H), slice(3 * H, 4 * H))
    b = b.copy()
    bi, bj, bf, bo = b[i_s].copy(), b[j_s].copy(), b[f_s].copy(), b[o_s].copy()
    bf += 1.0   # forget bias
    Wx, Wh = K[0:D], K[D : D + H]
    wxif = np.concatenate([Wx[:, f_s], Wx[:, i_s]], axis=1)
    wxjo = np.concatenate([Wx[:, o_s], Wx[:, j_s]], axis=1)
    whif = np.concatenate([Wh[:, f_s], Wh[:, i_s]], axis=1)
    whjo = np.concatenate([Wh[:, o_s], Wh[:, j_s]], axis=1)
    bif = np.concatenate([bf, bi])[None, :]
    bjo = np.concatenate([bo, bj])[None, :]
    whbif = np.concatenate([whif, bif], axis=0)
    whbjo = np.concatenate([whjo, bjo], axis=0)
    decwb = np.concatenate([np.asarray(dec_w, np.float32), np.asarray(dec_b, np.float32)[None, :]], axis=0)
    return (
        wxif.astype(BF16), wxjo.astype(BF16),
        whbif.astype(BF16), whbjo.astype(BF16), decwb.astype(BF16),
    )


def kernel(obss, lstm_kernel, lstm_bias, dec_w, dec_b, _nc_cache={}):
    obss = np.asarray(obss)
    wxif, wxjo, whbif, whbjo, decwb = prep_weights(lstm_kernel, lstm_bias, dec_w, dec_b)
    ob16 = obss.astype(BF16)

    if "nc" not in _nc_cache:
        _nc_cache["nc"] = build_nc()
    nc = _nc_cache["nc"]

    in_maps = []
    for i in range(NCORES):
        in_maps.append({
            "obss": ob16[i * BL : (i + 1) * BL],
            "wxif": wxif, "wxjo": wxjo, "whbif": whbif, "whbjo": whbjo,
            "decwb": decwb,
        })
    try:
        res = run_bass_kernel_spmd(nc, in_maps, core_ids=list(range(NCORES)))
    except Exception:
        # transient NRT_EXEC_UNIT_UNRECOVERABLE states clear on the next run
        res = run_bass_kernel_spmd(nc, in_maps, core_ids=list(range(NCORES)))
    outs = [res.results[i]["out"] for i in range(NCORES)]
    return np.concatenate(outs, axis=0).astype(np.float32)


if __name__ == "__main__":
    rng = np.random.default_rng(0)
    inputs = {
        "obss": rng.standard_normal((B, T, D), dtype=np.float32),
        "lstm_kernel": (rng.standard_normal((D + H, 4 * H)) * 0.1).astype(np.float32),
        "lstm_bias": np.zeros(4 * H, np.float32),
        "dec_w": (rng.standard_normal((H, A)) * 0.1).astype(np.float32),
        "dec_b": (rng.standard_normal(A) * 0.1).astype(np.float32),
    }
    out = kernel(**inputs)
    print("out", out.shape, out.dtype, out[0, 0, :4])



# revision 5
# speedup vs baseline: 1.2051x; 1.2051x over previous
"""Trainium2 Bass kernel: batch-512 LSTM (H=64, D=128, T=1024) + tanh decoder.

Strategy: data-parallel over batch across 8 NeuronCores (64 rows each).
Per core, transposed-state layout: state hT/c are [H, B] tiles, gates land in
one PSUM bank [128, 2B] (col-half 0 = (f,i), col-half 1 = (o,j)); one sigmoid
activation covers f/i/o (the j-quadrant sigmoid output is unused) and a second
small activation computes tanh(j) straight from PSUM — both live in the same
ACT table set so there is a single table load. Biases ride in via an augmented
ones-row on the h-side matmul (K=65). The decoder matmul for step t is emitted
after step t+1's h-matmuls so it stays off the recurrence critical path;
decoder outputs accumulate 32 steps per PSUM bank, then one batched tanh
writes the f32 staging tile. Input x is transposed to [D, t, B] by a single
per-chunk DMA-xbar transpose (dma_start_transpose) straight from DRAM (bf16),
costing no compute-engine time. All recurrence elementwise runs in bf16
(verified end-to-end rel err ~8e-3 vs the f32 reference).
Measured on silicon: ~2.39us/step steady state. v2: decoder tanh emitted
after the step's chain ops (fills the ACT idle window after tanh(c) instead
of delaying the next gate sigmoid, -690ns per 32 steps), DEC_BLK=16, TC=32
(4x smaller first-chunk DMA -> faster start, smaller tail).
"""
import sys

sys.path.insert(0, "/opt/trn_rl_repo")

import numpy as np
import ml_dtypes

import concourse.bass as bass
import concourse.bacc as bacc
import concourse.mybir as mybir
from concourse.tile import TileContext
from concourse.bass_utils import run_bass_kernel_spmd

BF16 = ml_dtypes.bfloat16
F32 = mybir.dt.float32
FB = mybir.dt.bfloat16
AF = mybir.ActivationFunctionType
OP = mybir.AluOpType

B, T, D, H, A = 512, 1024, 128, 64, 16
NCORES = 8
BL = B // NCORES  # 64 batch rows per core
TC = 32           # timesteps per chunk (small first chunk -> fast start)
DEC_BLK = 16      # timesteps per decoder PSUM bank (16*16 = 256 f32)

C_DT = FB         # cell-state dtype (bf16 verified: end-to-end rel err ~8e-3)
NG = 1            # interleaved batch groups per core (latency hiding)
GP_OFFLOAD = False # run m1/v4 on GpSimd to unload the vector engine


def build_nc(t_total=T):
    nc = bacc.Bacc()
    obss = nc.declare_dram_parameter("obss", [BL, T, D], FB, isOutput=False)
    wxif_d = nc.declare_dram_parameter("wxif", [D, 2 * H], FB, isOutput=False)
    wxjo_d = nc.declare_dram_parameter("wxjo", [D, 2 * H], FB, isOutput=False)
    whbif_d = nc.declare_dram_parameter("whbif", [H + 1, 2 * H], FB, isOutput=False)
    whbjo_d = nc.declare_dram_parameter("whbjo", [H + 1, 2 * H], FB, isOutput=False)
    decwb_d = nc.declare_dram_parameter("decwb", [H + 1, A], FB, isOutput=False)
    out = nc.declare_dram_parameter("out", [BL, T, A], F32, isOutput=True)

    with TileContext(nc) as tc:
        with (
            tc.tile_pool(name="const", bufs=1) as cpool,
            tc.tile_pool(name="state", bufs=1) as spool,
            tc.tile_pool(name="xT", bufs=2) as xpool,
            tc.tile_pool(name="stage", bufs=2) as stpool,
            tc.tile_pool(name="work", bufs=3) as wpool,
            tc.tile_pool(name="psz", bufs=2, space="PSUM") as pzpool,
            tc.tile_pool(name="psd", bufs=2, space="PSUM") as pdpool,
        ):
            # all tiles allocated 128-partition so every base partition is 0
            # (2-input DVE ops require equal input base partitions)
            wxif = cpool.tile([D, 2 * H], FB, tag="wxif")
            wxjo = cpool.tile([D, 2 * H], FB, tag="wxjo")
            whbif_t = cpool.tile([D, 2 * H], FB, tag="whbif")
            whbjo_t = cpool.tile([D, 2 * H], FB, tag="whbjo")
            decwb_t = cpool.tile([D, A], FB, tag="decwb")
            whbif = whbif_t[0 : H + 1, :]
            whbjo = whbjo_t[0 : H + 1, :]
            decwb = decwb_t[0 : H + 1, :]
            nc.sync.dma_start(wxif[:, :], wxif_d[:, :])
            nc.sync.dma_start(wxjo[:, :], wxjo_d[:, :])
            nc.sync.dma_start(whbif, whbif_d[:, :])
            nc.sync.dma_start(whbjo, whbjo_d[:, :])
            nc.sync.dma_start(decwb, decwb_d[:, :])

            BG = BL // NG  # batch rows per group
            hTs, csts = [], []
            for g in range(NG):
                hT_t = spool.tile([D, BG], FB, tag=f"hT{g}")
                cst_t = spool.tile([D, BG], C_DT, tag=f"c{g}")
                nc.vector.memset(hT_t[0:H, :], 0.0)
                nc.vector.memset(hT_t[H : H + 1, :], 1.0)
                nc.vector.memset(cst_t[0:H, :], 0.0)
                hTs.append(hT_t)
                csts.append(cst_t)

            n_chunks = t_total // TC
            dec_state = {}

            for ch in range(n_chunks):
                t0 = ch * TC
                xT = xpool.tile([D, TC * BL], FB, tag="xT")
                nc.sync.dma_start_transpose(
                    xT[:, :].rearrange("d (t b) -> d t b", t=TC),
                    obss[:, t0 : t0 + TC, :].rearrange("b t d -> b (t d)"),
                )
                stage = stpool.tile([BL, TC * A], F32, tag="stage")
                stage_ref = {"stage": stage}

                def emit_dec(td, g, stage_ref=stage_ref):
                    # decoder matmul for step td, group g: out[b, A] = h @ dec_w
                    # + dec_b via the ones-row of hT (batched tanh emitted
                    # separately, off the ACT critical path)
                    if td < 0:
                        return
                    dcol = td % DEC_BLK
                    if dcol == 0 and g == 0:
                        psd_tile = pdpool.tile([BL, DEC_BLK * A], F32, tag="psd")
                        dec_state["psd"] = psd_tile
                    psd = dec_state["psd"]
                    nc.tensor.matmul(
                        psd[g * BG : (g + 1) * BG, dcol * A : (dcol + 1) * A],
                        hTs[g][0 : H + 1, :], decwb, start=True, stop=True,
                    )

                def emit_dec_tanh(td, stage_ref=stage_ref):
                    # batched decoder tanh for the DEC_BLK block ending at td.
                    # Emitted at the END of a step's ACT queue so it fills the
                    # ACT idle window after tanh(c) instead of delaying the
                    # next step's gate sigmoid (which costs ~690ns/occurrence).
                    if td < 0 or (td % DEC_BLK) != DEC_BLK - 1:
                        return
                    blk = td // DEC_BLK
                    nc.scalar.activation(
                        stage_ref["stage"][:, blk * DEC_BLK * A : (blk + 1) * DEC_BLK * A],
                        dec_state["psd"][:, :], AF.Tanh,
                    )

                for tt in range(TC):
                    for g in range(NG):
                        hT = hTs[g][0 : H + 1, :]
                        cst = csts[g][0:H, :]
                        xcol = xT[:, tt * BL + g * BG : tt * BL + (g + 1) * BG]
                        # two PSUM banks: the (f,i) sigmoid only waits for the
                        # if-half matmuls, starting ~1 matmul earlier; sigma(o)
                        # comes last and is off the critical path (only v4 needs it)
                        psz_if = pzpool.tile([2 * H, BG], F32, tag=f"pszif{g}")
                        psz_jo = pzpool.tile([2 * H, BG], F32, tag=f"pszjo{g}")
                        nc.tensor.matmul(psz_if[:, :], wxif[:, :], xcol, start=True, stop=False)
                        nc.tensor.matmul(psz_if[:, :], whbif, hT, start=False, stop=True)
                        nc.tensor.matmul(psz_jo[:, :], wxjo[:, :], xcol, start=True, stop=False)
                        nc.tensor.matmul(psz_jo[:, :], whbjo, hT, start=False, stop=True)

                        # decoder for the PREVIOUS step, emitted here so the PE
                        # runs it after this step's h-matmuls -> off the chain
                        emit_dec(tt - 1, g)

                        # gate partition layout: if-bank rows = (f; i), jo-bank
                        # rows = (o; j) — f/o at base partition 0, i/j at base 64,
                        # so every 2-input DVE op pairs operands with equal bases
                        s = wpool.tile([2 * H, BG], FB, tag=f"s{g}")
                        nc.scalar.activation(s[:, :], psz_if[:, :], AF.Sigmoid)
                        tj_t = wpool.tile([D, BG], FB, tag=f"tj{g}")
                        tj = tj_t[H : 2 * H, :]
                        nc.scalar.activation(tj, psz_jo[H : 2 * H, :], AF.Tanh)
                        so_t = wpool.tile([D, BG], FB, tag=f"so{g}")
                        so = so_t[0:H, :]
                        nc.scalar.activation(so, psz_jo[0:H, :], AF.Sigmoid)
                        sf = s[0:H, :]
                        si = s[H : 2 * H, :]

                        cf_t = wpool.tile([D, BG], C_DT, tag=f"cf{g}")
                        cf = cf_t[0:H, :]
                        nc.vector.tensor_mul(cf, cst, sf)
                        u_t = wpool.tile([D, BG], FB, tag=f"u{g}")
                        u = u_t[0:H, :]
                        nc.vector.tensor_mul(u, tj, si)
                        nc.vector.tensor_add(cst, cf, u)
                        tch_t = wpool.tile([D, BG], FB, tag=f"tch{g}")
                        tch = tch_t[0:H, :]
                        nc.scalar.activation(tch, cst, AF.Tanh)
                        (nc.gpsimd if GP_OFFLOAD else nc.vector).tensor_mul(hTs[g][0:H, :], tch, so)
                    # decoder tanh for the block ending at tt-1, placed after
                    # this step's chain ops in the ACT queue
                    emit_dec_tanh(tt - 1)
                for g in range(NG):
                    emit_dec(TC - 1, g)
                emit_dec_tanh(TC - 1)
                nc.sync.dma_start(out[:, t0 : t0 + TC, :], stage[:, :])
    nc.finalize()
    return nc


def prep_weights(lstm_kernel, lstm_bias, dec_w, dec_b):
    K = np.asarray(lstm_kernel, np.float32)
    b = np.asarray(lstm_bias, np.float32).copy()
    i_s, j_s, f_s, o_s = (slice(0, H), slice(H, 2 * H), slice(2 * H, 3 * H), slice(3 * H, 4 * H))
    b = b.copy()
    bi, bj, bf, bo = b[i_s].copy(), b[j_s].copy(), b[f_s].copy(), b[o_s].copy()
    bf += 1.0   # forget bias
    Wx, Wh = K[0:D], K[D : D + H]
    wxif = np.concatenate([Wx[:, f_s], Wx[:, i_s]], axis=1)
    wxjo = np.concatenate([Wx[:, o_s], Wx[:, j_s]], axis=1)
    whif = np.concatenate([Wh[:, f_s], Wh[:, i_s]], axis=1)
    whjo = np.concatenate([Wh[:, o_s], Wh[:, j_s]], axis=1)
    bif = np.concatenate([bf, bi])[None, :]
    bjo = np.concatenate([bo, bj])[None, :]
    whbif = np.concatenate([whif, bif], axis=0)
    whbjo = np.concatenate([whjo, bjo], axis=0)
    decwb = np.concatenate([np.asarray(dec_w, np.float32), np.asarray(dec_b, np.float32)[None, :]], axis=0)
    return (
        wxif.astype(BF16), wxjo.astype(BF16),
        whbif.astype(BF16), whbjo.astype(BF16), decwb.astype(BF16),
    )


def kernel(obss, lstm_kernel, lstm_bias, dec_w, dec_b, _nc_cache={}):
    obss = np.asarray(obss)
    wxif, wxjo, whbif, whbjo, decwb = prep_weights(lstm_kernel, lstm_bias, dec_w, dec_b)
    ob16 = obss.astype(BF16)

    if "nc" not in _nc_cache:
        _nc_cache["nc"] = build_nc()
    nc = _nc_cache["nc"]

    in_maps = []
    for i in range(NCORES):
        in_maps.append({
            "obss": ob16[i * BL : (i + 1) * BL],
            "wxif": wxif, "wxjo": wxjo, "whbif": whbif, "whbjo": whbjo,
            "decwb": decwb,
        })
    try:
        res = run_bass_kernel_spmd(nc, in_maps, core_ids=list(range(NCORES)))
    except Exception:
        # transient NRT_EXEC_UNIT_UNRECOVERABLE states clear on the next run
        res = run_bass_kernel_spmd(nc, in_maps, core_ids=list(range(NCORES)))
    outs = [res.results[i]["out"] for i in range(NCORES)]
    return np.concatenate(outs, axis=0).astype(np.float32)


if __name__ == "__main__":
    rng = np.random.default_rng(0)
    inputs = {
        "obss": rng.standard_normal((B, T, D), dtype=np.float32),
        "lstm_kernel": (rng.standard_normal((D + H, 4 * H)) * 0.1).astype(np.float32),
        "lstm_bias": np.zeros(4 * H, np.float32),
        "dec_w": (rng.standard_normal((H, A)) * 0.1).astype(np.float32),
        "dec_b": (rng.standard_normal(A) * 0.1).astype(np.float32),
    }
    out = kernel(**inputs)
    print("out", out.shape, out.dtype, out[0, 0, :4])



# revision 9
# speedup vs baseline: 1.2069x; 1.0015x over previous
"""Trainium2 Bass kernel: batch-512 LSTM (H=64, D=128, T=1024) + tanh decoder.

Strategy: data-parallel over batch across 8 NeuronCores (64 rows each).
Per core, transposed-state layout: state hT/c are [H, B] tiles, gates land in
one PSUM bank [128, 2B] (col-half 0 = (f,i), col-half 1 = (o,j)); one sigmoid
activation covers f/i/o (the j-quadrant sigmoid output is unused) and a second
small activation computes tanh(j) straight from PSUM — both live in the same
ACT table set so there is a single table load. Biases ride in via an augmented
ones-row on the h-side matmul (K=65). The decoder matmul for step t is emitted
after step t+1's h-matmuls so it stays off the recurrence critical path;
decoder outputs accumulate 32 steps per PSUM bank, then one batched tanh
writes the f32 staging tile. Input x is transposed to [D, t, B] by a single
per-chunk DMA-xbar transpose (dma_start_transpose) straight from DRAM (bf16),
costing no compute-engine time. All recurrence elementwise runs in bf16
(verified end-to-end rel err ~8e-3 vs the f32 reference).
Measured on silicon: ~2.39us/step steady state. v2: decoder tanh emitted
after the step's chain ops (fills the ACT idle window after tanh(c) instead
of delaying the next gate sigmoid, -690ns per 32 steps), DEC_BLK=16, TC=32
(4x smaller first-chunk DMA -> faster start, smaller tail).
"""
import sys

sys.path.insert(0, "/opt/trn_rl_repo")

import numpy as np
import ml_dtypes

import concourse.bass as bass
import concourse.bacc as bacc
import concourse.mybir as mybir
from concourse.tile import TileContext
from concourse.bass_utils import run_bass_kernel_spmd

BF16 = ml_dtypes.bfloat16
F32 = mybir.dt.float32
FB = mybir.dt.bfloat16
AF = mybir.ActivationFunctionType
OP = mybir.AluOpType

B, T, D, H, A = 512, 1024, 128, 64, 16
NCORES = 8
BL = B // NCORES  # 64 batch rows per core
TC = 32           # timesteps per chunk (small first chunk -> fast start)
DEC_BLK = 16      # timesteps per decoder PSUM bank (16*16 = 256 f32)

C_DT = FB         # cell-state dtype (bf16 verified: end-to-end rel err ~8e-3)
NG = 1            # interleaved batch groups per core (latency hiding)
GP_OFFLOAD = False # run m1/v4 on GpSimd to unload the vector engine


def build_nc(t_total=T):
    nc = bacc.Bacc()
    # obss arrives HOST-PRE-TRANSPOSED as [D, T, BL]: the per-chunk load is
    # then a fat contiguous DMA (4KB/partition) instead of an element-
    # scattered dma_start_transpose that kept the DMA engines busy >50% of
    # the run and contended with ACT/DVE SBUF ports.
    obss = nc.declare_dram_parameter("obss", [D, T, BL], FB, isOutput=False)
    wxif_d = nc.declare_dram_parameter("wxif", [D, 2 * H], FB, isOutput=False)
    wxjo_d = nc.declare_dram_parameter("wxjo", [D, 2 * H], FB, isOutput=False)
    whbif_d = nc.declare_dram_parameter("whbif", [H + 1, 2 * H], FB, isOutput=False)
    whbjo_d = nc.declare_dram_parameter("whbjo", [H + 1, 2 * H], FB, isOutput=False)
    decwb_d = nc.declare_dram_parameter("decwb", [H + 1, A], FB, isOutput=False)
    out = nc.declare_dram_parameter("out", [BL, T, A], F32, isOutput=True)

    with TileContext(nc) as tc:
        with (
            tc.tile_pool(name="const", bufs=1) as cpool,
            tc.tile_pool(name="state", bufs=1) as spool,
            tc.tile_pool(name="xT", bufs=2) as xpool,
            tc.tile_pool(name="stage", bufs=2) as stpool,
            tc.tile_pool(name="work", bufs=3) as wpool,
            tc.tile_pool(name="psz", bufs=2, space="PSUM") as pzpool,
            tc.tile_pool(name="psd", bufs=2, space="PSUM") as pdpool,
        ):
            # all tiles allocated 128-partition so every base partition is 0
            # (2-input DVE ops require equal input base partitions)
            wxif = cpool.tile([D, 2 * H], FB, tag="wxif")
            wxjo = cpool.tile([D, 2 * H], FB, tag="wxjo")
            whbif_t = cpool.tile([D, 2 * H], FB, tag="whbif")
            whbjo_t = cpool.tile([D, 2 * H], FB, tag="whbjo")
            decwb_t = cpool.tile([D, A], FB, tag="decwb")
            whbif = whbif_t[0 : H + 1, :]
            whbjo = whbjo_t[0 : H + 1, :]
            decwb = decwb_t[0 : H + 1, :]
            nc.sync.dma_start(wxif[:, :], wxif_d[:, :])
            nc.sync.dma_start(wxjo[:, :], wxjo_d[:, :])
            nc.sync.dma_start(whbif, whbif_d[:, :])
            nc.sync.dma_start(whbjo, whbjo_d[:, :])
            nc.sync.dma_start(decwb, decwb_d[:, :])

            BG = BL // NG  # batch rows per group
            hTs, csts = [], []
            for g in range(NG):
                hT_t = spool.tile([D, BG], FB, tag=f"hT{g}")
                cst_t = spool.tile([D, BG], C_DT, tag=f"c{g}")
                nc.vector.memset(hT_t[0:H, :], 0.0)
                nc.vector.memset(hT_t[H : H + 1, :], 1.0)
                nc.vector.memset(cst_t[0:H, :], 0.0)
                hTs.append(hT_t)
                csts.append(cst_t)

            n_chunks = t_total // TC
            dec_state = {}

            for ch in range(n_chunks):
                t0 = ch * TC
                xT = xpool.tile([D, TC * BL], FB, tag="xT")
                nc.sync.dma_start(
                    xT[:, :],
                    obss[:, t0 : t0 + TC, :].rearrange("d t b -> d (t b)"),
                )
                stage = stpool.tile([BL, TC * A], F32, tag="stage")
                stage_ref = {"stage": stage}

                def emit_dec(td, g, stage_ref=stage_ref):
                    # decoder matmul for step td, group g: out[b, A] = h @ dec_w
                    # + dec_b via the ones-row of hT (batched tanh emitted
                    # separately, off the ACT critical path)
                    if td < 0:
                        return
                    dcol = td % DEC_BLK
                    if dcol == 0 and g == 0:
                        psd_tile = pdpool.tile([BL, DEC_BLK * A], F32, tag="psd")
                        dec_state["psd"] = psd_tile
                    psd = dec_state["psd"]
                    nc.tensor.matmul(
                        psd[g * BG : (g + 1) * BG, dcol * A : (dcol + 1) * A],
                        hTs[g][0 : H + 1, :], decwb, start=True, stop=True,
                    )

                def emit_dec_tanh(td, stage_ref=stage_ref):
                    # batched decoder tanh for the DEC_BLK block ending at td.
                    # Emitted at the END of a step's ACT queue so it fills the
                    # ACT idle window after tanh(c) instead of delaying the
                    # next step's gate sigmoid (which costs ~690ns/occurrence).
                    if td < 0 or (td % DEC_BLK) != DEC_BLK - 1:
                        return
                    blk = td // DEC_BLK
                    nc.scalar.activation(
                        stage_ref["stage"][:, blk * DEC_BLK * A : (blk + 1) * DEC_BLK * A],
                        dec_state["psd"][:, :], AF.Tanh,
                    )

                for tt in range(TC):
                    # OP-INTERLEAVED emission across the NG independent batch-
                    # group chains: engines execute their queues in order, so
                    # group-by-group emission would lockstep chain 1 a full
                    # period behind chain 0. Interleaving at op granularity
                    # keeps the stagger at ~one op, letting both chains run
                    # concurrently with per-op column counts halved.
                    psz_if, psz_jo = {}, {}
                    for g in range(NG):
                        psz_if[g] = pzpool.tile([2 * H, BG], F32, tag=f"pszif{g}")
                        psz_jo[g] = pzpool.tile([2 * H, BG], F32, tag=f"pszjo{g}")
                    # PE: x-part matmuls first (no h dependency), then the
                    # h-matmuls per group in chain order, then decoders
                    for g in range(NG):
                        xcol = xT[:, tt * BL + g * BG : tt * BL + (g + 1) * BG]
                        nc.tensor.matmul(psz_if[g][:, :], wxif[:, :], xcol, start=True, stop=False)
                        nc.tensor.matmul(psz_jo[g][:, :], wxjo[:, :], xcol, start=True, stop=False)
                    for g in range(NG):
                        hT = hTs[g][0 : H + 1, :]
                        nc.tensor.matmul(psz_if[g][:, :], whbif, hT, start=False, stop=True)
                        nc.tensor.matmul(psz_jo[g][:, :], whbjo, hT, start=False, stop=True)
                    for g in range(NG):
                        emit_dec(tt - 1, g)

                    # gate partition layout: if-bank rows = (f; i), jo-bank
                    # rows = (o; j) — f/o at base partition 0, i/j at base 64,
                    # so every 2-input DVE op pairs operands with equal bases
                    s, tj, so, tch = {}, {}, {}, {}
                    for g in range(NG):
                        s[g] = wpool.tile([2 * H, BG], FB, tag=f"s{g}")
                        nc.scalar.activation(s[g][:, :], psz_if[g][:, :], AF.Sigmoid)
                        tj_t = wpool.tile([D, BG], FB, tag=f"tj{g}")
                        tj[g] = tj_t[H : 2 * H, :]
                        nc.scalar.activation(tj[g], psz_jo[g][H : 2 * H, :], AF.Tanh)
                    for g in range(NG):
                        so_t = wpool.tile([D, BG], FB, tag=f"so{g}")
                        so[g] = so_t[0:H, :]
                        nc.scalar.activation(so[g], psz_jo[g][0:H, :], AF.Sigmoid)
                    for g in range(NG):
                        cst = csts[g][0:H, :]
                        cf_t = wpool.tile([D, BG], C_DT, tag=f"cf{g}")
                        cf = cf_t[0:H, :]
                        nc.vector.tensor_mul(cf, cst, s[g][0:H, :])
                        u_t = wpool.tile([D, BG], FB, tag=f"u{g}")
                        u = u_t[0:H, :]
                        nc.vector.tensor_mul(u, tj[g], s[g][H : 2 * H, :])
                        nc.vector.tensor_add(cst, cf, u)
                    for g in range(NG):
                        tch_t = wpool.tile([D, BG], FB, tag=f"tch{g}")
                        tch[g] = tch_t[0:H, :]
                        nc.scalar.activation(tch[g], csts[g][0:H, :], AF.Tanh)
                    for g in range(NG):
                        (nc.gpsimd if GP_OFFLOAD else nc.vector).tensor_mul(hTs[g][0:H, :], tch[g], so[g])
                    # decoder tanh for the block ending at tt-1, placed after
                    # this step's chain ops in the ACT queue
                    emit_dec_tanh(tt - 1)
                for g in range(NG):
                    emit_dec(TC - 1, g)
                emit_dec_tanh(TC - 1)
                nc.sync.dma_start(out[:, t0 : t0 + TC, :], stage[:, :])
    nc.finalize()
    return nc


def prep_weights(lstm_kernel, lstm_bias, dec_w, dec_b):
    K = np.asarray(lstm_kernel, np.float32)
    b = np.asarray(lstm_bias, np.float32).copy()
    i_s, j_s, f_s, o_s = (slice(0, H), slice(H, 2 * H), slice(2 * H, 3 * H), slice(3 * H, 4 * H))
    b = b.copy()
    bi, bj, bf, bo = b[i_s].copy(), b[j_s].copy(), b[f_s].copy(), b[o_s].copy()
    bf += 1.0   # forget bias
    Wx, Wh = K[0:D], K[D : D + H]
    wxif = np.concatenate([Wx[:, f_s], Wx[:, i_s]], axis=1)
    wxjo = np.concatenate([Wx[:, o_s], Wx[:, j_s]], axis=1)
    whif = np.concatenate([Wh[:, f_s], Wh[:, i_s]], axis=1)
    whjo = np.concatenate([Wh[:, o_s], Wh[:, j_s]], axis=1)
    bif = np.concatenate([bf, bi])[None, :]
    bjo = np.concatenate([bo, bj])[None, :]
    whbif = np.concatenate([whif, bif], axis=0)
    whbjo = np.concatenate([whjo, bjo], axis=0)
    decwb = np.concatenate([np.asarray(dec_w, np.float32), np.asarray(dec_b, np.float32)[None, :]], axis=0)
    return (
        wxif.astype(BF16), wxjo.astype(BF16),
        whbif.astype(BF16), whbjo.astype(BF16), decwb.astype(BF16),
    )


def make_in_maps(obss, lstm_kernel, lstm_bias, dec_w, dec_b):
    wxif, wxjo, whbif, whbjo, decwb = prep_weights(lstm_kernel, lstm_bias, dec_w, dec_b)
    ob16 = np.asarray(obss).astype(BF16)
    in_maps = []
    for i in range(NCORES):
        # host-side transpose to [D, T, BL] (see build_nc comment)
        obT = np.ascontiguousarray(ob16[i * BL : (i + 1) * BL].transpose(2, 1, 0))
        in_maps.append({
            "obss": obT,
            "wxif": wxif, "wxjo": wxjo, "whbif": whbif, "whbjo": whbjo,
            "decwb": decwb,
        })
    return in_maps


def kernel(obss, lstm_kernel, lstm_bias, dec_w, dec_b, _nc_cache={}):
    if "nc" not in _nc_cache:
        _nc_cache["nc"] = build_nc()
    nc = _nc_cache["nc"]

    in_maps = make_in_maps(obss, lstm_kernel, lstm_bias, dec_w, dec_b)
    try:
        res = run_bass_kernel_spmd(nc, in_maps, core_ids=list(range(NCORES)))
    except Exception:
        # transient NRT_EXEC_UNIT_UNRECOVERABLE states clear on the next run
        res = run_bass_kernel_spmd(nc, in_maps, core_ids=list(range(NCORES)))
    outs = [res.results[i]["out"] for i in range(NCORES)]
    return np.concatenate(outs, axis=0).astype(np.float32)


if __name__ == "__main__":
    rng = np.random.default_rng(0)
    inputs = {
        "obss": rng.standard_normal((B, T, D), dtype=np.float32),
        "lstm_kernel": (rng.standard_normal((D + H, 4 * H)) * 0.1).astype(np.float32),
        "lstm_bias": np.zeros(4 * H, np.float32),
        "dec_w": (rng.standard_normal((H, A)) * 0.1).astype(np.float32),
        "dec_b": (rng.standard_normal(A) * 0.1).astype(np.float32),
    }
    out = kernel(**inputs)
    print("out", out.shape, out.dtype, out[0, 0, :4])

